# revision 14
# baseline (speedup 1.0000x reference)
"""Bahdanau attention kernel for Trainium2, 8-core SPMD data-parallel over batch.

Problem shapes (hardcoded): values [32,2048,512] f32, query [32,512],
W1/W2 [512,512], b1/b2 [512], V [512,1], bV [1].

reference:
    proj_v = values @ W1 + b1            [B,T,U]
    proj_q = (query @ W2 + b2)[:,None,:] [B,1,U]
    score  = tanh(proj_v+proj_q) @ V + bV
    a      = softmax(score, axis=T)
    ctx    = sum_t a * values            [B,D]
    return ctx, a

Per-core plan (4 batches each):
  - load values[b] natural [t,d] -> SBUF (fast contiguous DMA)
  - PE-transpose 128x128 tiles -> vT [d,t] (PSUM->SBUF via DVE copies)
  - mm1: pv^T[u,t] = sum_d W1[d,u] * vT[d,t]  (fp32r, N=512, full PE rate)
  - ACT: s[u,t] = tanh(pv^T + beta[u,b]) where beta = (q@W2)+b1+b2 per batch
  - V-dot: score[1,t] = sum_u V[u]*s[u,t] (M=1 matmuls, accumulate over u-chunks)
  - ACT: e = exp(score) row-wise, with fused accumulated Z partials
  - PE-transpose e row -> e column layout [128,16]
  - context: sum_tc ecol[:,tc].T @ vnat[:,tc,:] -> [1,512]; scale by 1/Z
  - bV is skipped: softmax is shift-invariant, so it cancels exactly.
"""

import sys

if "/opt/trn_rl_repo" not in sys.path:
    sys.path.insert(0, "/opt/trn_rl_repo")

from contextlib import ExitStack

import numpy as np

import concourse.bacc as bacc
import concourse.tile as tile
from concourse import mybir
from concourse.bass_utils import run_bass_kernel_spmd
from concourse.masks import make_identity

F32 = mybir.dt.float32
MMDT = mybir.dt.float32r  # streaming dtype for matmuls (full PE rate at N>=256)
AF = mybir.ActivationFunctionType

B, T, D, U = 32, 2048, 512, 512
NCORES = 8
BL = B // NCORES  # batches per core
P = 128
DCH = D // P  # 4 d-chunks
UCH = U // P  # 4 u-chunks
TCH = T // P  # 16 t-chunks
TGS = 512     # matmul free-dim block over t
TG = T // TGS  # 4 t-blocks

_CACHE = {}


def _mm(ap):
    return ap.bitcast(MMDT)


def build(debug=False):
    nc = bacc.Bacc("TRN2", target_bir_lowering=False, debug=debug)

    vals = nc.dram_tensor("values_l", [BL, T, D], F32, kind="ExternalInput").ap()
    qry = nc.dram_tensor("query_l", [BL, D], F32, kind="ExternalInput").ap()
    w1 = nc.dram_tensor("W1", [D, U], F32, kind="ExternalInput").ap()
    w2 = nc.dram_tensor("W2", [D, U], F32, kind="ExternalInput").ap()
    b1 = nc.dram_tensor("b1", [U], F32, kind="ExternalInput").ap()
    b2 = nc.dram_tensor("b2", [U], F32, kind="ExternalInput").ap()
    vv = nc.dram_tensor("V", [U, 1], F32, kind="ExternalInput").ap()
    ctx_out = nc.dram_tensor("ctx_out", [BL, D], F32, kind="ExternalOutput").ap()
    attn_out = nc.dram_tensor("attn_out", [BL, T], F32, kind="ExternalOutput").ap()
    z_out = nc.dram_tensor("z_out", [1, BL], F32, kind="ExternalOutput").ap()

    with ExitStack() as ctx:
        tc = ctx.enter_context(tile.TileContext(nc))

        consts = ctx.enter_context(tc.tile_pool(name="consts", bufs=1))
        vnat_p = ctx.enter_context(tc.tile_pool(name="vnat", bufs=2))
        vT_p = ctx.enter_context(tc.tile_pool(name="vT", bufs=2))
        s_p = ctx.enter_context(tc.tile_pool(name="s", bufs=1))
        small = ctx.enter_context(tc.tile_pool(name="small", bufs=2))
        ptr = ctx.enter_context(tc.tile_pool(name="ptr", bufs=2, space="PSUM"))
        ppv = ctx.enter_context(tc.tile_pool(name="ppv", bufs=4, space="PSUM"))
        psc = ctx.enter_context(tc.tile_pool(name="psc", bufs=2, space="PSUM"))

        ident = consts.tile([P, P], F32)
        make_identity(nc, ident)
        identr = consts.tile([P, P], MMDT)
        nc.vector.tensor_copy(identr, ident)

        # warm-up matmuls: transpose-mode does not engage the PE clock-gate
        # (HAM), so run real matmuls while the first values DMA lands to have
        # the array at 2.4GHz when mm1 starts
        warm = ptr.tile([P, P], F32, tag="tr", name="warm")
        for i in range(12):
            nc.tensor.matmul(warm, identr, identr, start=(i == 0), stop=(i == 11))

        # small consts first (cheap DMA issues; setup matmuls need them early)
        qsb = consts.tile([BL, D], F32)
        nc.sync.dma_start(out=qsb, in_=qry)
        b1row = consts.tile([1, U], F32)
        nc.sync.dma_start(out=b1row, in_=b1.rearrange("(one u) -> one u", one=1))
        b2row = consts.tile([1, U], F32)
        nc.sync.dma_start(out=b2row, in_=b2.rearrange("(one u) -> one u", one=1))
        b12row = consts.tile([1, U], F32)
        nc.vector.tensor_add(b12row, b1row, b2row)
        vrow = consts.tile([1, U], F32)
        nc.sync.dma_start(out=vrow, in_=vv.rearrange("u one -> one u"))

        def load_vnat(b, name):
            # chunked so the first transposes can start before the whole 4MB lands
            vnat = vnat_p.tile([P, TCH, D], MMDT, tag="vnat", name=name)
            src_ap = vals[b].rearrange("(t p) d -> p t d", p=P).bitcast(MMDT)
            for tg in range(TG):
                nc.sync.dma_start(
                    out=vnat[:, tg * 4 : (tg + 1) * 4, :],
                    in_=src_ap[:, tg * 4 : (tg + 1) * 4, :],
                )
            return vnat

        vnat0 = load_vnat(0, "vnat_first")

        # W1 loaded with an f32r-casting SWDGE DMA; W2 stays f32 for the
        # (tiny) f32 beta matmul.
        W1r = consts.tile([P, DCH, U], MMDT)
        nc.gpsimd.dma_start(out=W1r, in_=w1.rearrange("(dc p) u -> p dc u", p=P))
        W2sb = consts.tile([P, DCH, U], F32)
        nc.sync.dma_start(out=W2sb, in_=w2.rearrange("(dc p) u -> p dc u", p=P))

        Vsb = consts.tile([P, UCH], MMDT)
        b12c = consts.tile([P, UCH], F32)
        qTc = consts.tile([P, DCH, BL], F32)
        beta = consts.tile([P, UCH, BL], F32)
        zall = consts.tile([1, BL], F32)

        def setup_small():
            # V, b1+b2, q transposed to column layouts; beta = (q@W2)^T + b1 + b2.
            ps_v = ptr.tile([P, TGS], F32, tag="tr", name="ps_v")
            for i in range(UCH):
                nc.tensor.matmul(
                    ps_v[:, i : i + 1], vrow[0:1, i * P : (i + 1) * P],
                    ident[0:1, 0:1], is_transpose=True,
                    start=(i == 0), stop=(i == UCH - 1),
                )
            nc.vector.tensor_copy(Vsb, ps_v[:, 0:UCH])

            ps_b = ptr.tile([P, TGS], F32, tag="tr", name="ps_b")
            for i in range(UCH):
                nc.tensor.matmul(
                    ps_b[:, i : i + 1], b12row[0:1, i * P : (i + 1) * P],
                    ident[0:1, 0:1], is_transpose=True,
                    start=(i == 0), stop=(i == UCH - 1),
                )
            nc.vector.tensor_copy(b12c, ps_b[:, 0:UCH])

            ps_q = ptr.tile([P, TGS], F32, tag="tr", name="ps_q")
            for dc in range(DCH):
                nc.tensor.matmul(
                    ps_q[:, dc * BL : (dc + 1) * BL], qsb[0:BL, dc * P : (dc + 1) * P],
                    ident[0:BL, 0:BL], is_transpose=True,
                    start=(dc == 0), stop=(dc == DCH - 1),
                )
            nc.vector.tensor_copy(qTc, ps_q[:, 0 : DCH * BL].rearrange("p (dc b) -> p dc b", dc=DCH))

            for uc in range(UCH):
                pq = ppv.tile([P, BL], F32, tag="pv", name=f"pq{uc}")
                for dc in range(DCH):
                    nc.tensor.matmul(
                        pq, W2sb[:, dc, uc * P : (uc + 1) * P], qTc[:, dc, :],
                        start=(dc == 0), stop=(dc == DCH - 1),
                    )
                nc.vector.tensor_scalar_add(beta[:, uc, :], pq, b12c[:, uc : uc + 1])

        def tr_group(b, vnat, vT, tg, dc):
            # one 512-wide column of the values transpose: 4 PE transposes + copy
            tr = ptr.tile([P, TGS], MMDT, tag="tr", name=f"tr{b}_{dc}_{tg}")
            for t4 in range(4):
                ti = tg * 4 + t4
                nc.tensor.matmul(
                    tr[:, t4 * P : (t4 + 1) * P],
                    vnat[:, ti, dc * P : (dc + 1) * P],
                    identr, is_transpose=True,
                    start=(t4 == 0), stop=(t4 == 3),
                )
            nc.vector.tensor_copy(vT[:, dc, tg * TGS : (tg + 1) * TGS], tr)

        def mm1_group(b, s, vT, tg, uc):
            pv = ppv.tile([P, TGS], F32, tag="pv")
            for dc in range(DCH):
                nc.tensor.matmul(
                    pv, W1r[:, dc, uc * P : (uc + 1) * P],
                    vT[:, dc, tg * TGS : (tg + 1) * TGS],
                    start=(dc == 0), stop=(dc == DCH - 1),
                )
            nc.scalar.activation(
                s[:, uc, tg * TGS : (tg + 1) * TGS], pv, AF.Tanh,
                bias=beta[:, uc, b : b + 1],
            )

        def vdot_exp(b, s, e_row, zpart, tg):
            sc = psc.tile([1, TGS], F32, tag="sc", name=f"sc{b}_{tg}")
            for uc in range(UCH):
                nc.tensor.matmul(
                    sc, Vsb[:, uc : uc + 1],
                    s[:, uc, tg * TGS : (tg + 1) * TGS],
                    start=(uc == 0), stop=(uc == UCH - 1),
                )
            nc.scalar.activation(
                e_row[:, tg * TGS : (tg + 1) * TGS], sc, AF.Exp,
                accum_out=zpart[:, tg : tg + 1],
            )

        def etr_group(pecol, e_row, tg):
            # 4 e-row elements blocks -> column layout, one psum group per tg
            for j in range(4):
                ti = tg * 4 + j
                nc.tensor.matmul(
                    pecol[:, ti : ti + 1], e_row[0:1, ti * P : (ti + 1) * P],
                    ident[0:1, 0:1], is_transpose=True,
                    start=(j == 0), stop=(j == 3),
                )

        setup_small()

        vnat_cur = vnat0
        vT_cur = vT_p.tile([P, DCH, T], MMDT, tag="vT", name="vT_first")

        for b in range(BL):
            vnat, vT = vnat_cur, vT_cur
            nxt = b + 1 < BL
            if nxt:
                vnat_nxt = load_vnat(b + 1, f"vnat{b+1}")
                vT_nxt = vT_p.tile([P, DCH, T], MMDT, tag="vT", name=f"vT{b+1}")

            s = s_p.tile([P, UCH, T], MMDT, tag="s")
            e_row = small.tile([1, T], F32, tag="e_row", bufs=1)
            zpart = small.tile([1, TG], F32, tag="zpart")
            pecol = ppv.tile([P, TGS], F32, tag="pv", name=f"pecol{b}")

            # batch 0 transposes its own data chunk-by-chunk as the DMA lands;
            # later batches interleave the NEXT batch's transposes into mm1
            # (their LDWEIGHTS hide under mm1's long streams). V-dot runs one
            # tg behind mm1; e-transposes two behind (after exp).
            for tg in range(TG):
                if b == 0:
                    for dc in range(DCH):
                        tr_group(0, vnat, vT, tg, dc)
                for uc in range(UCH):
                    mm1_group(b, s, vT, tg, uc)
                    if nxt and b > 0:
                        tr_group(b + 1, vnat_nxt, vT_nxt, tg, uc)
                if tg >= 1:
                    vdot_exp(b, s, e_row, zpart, tg - 1)
                if tg >= 2:
                    etr_group(pecol, e_row, tg - 2)
            vdot_exp(b, s, e_row, zpart, TG - 1)
            etr_group(pecol, e_row, TG - 2)
            if b == 0 and nxt:
                for g in range(4):
                    tr_group(1, vnat_nxt, vT_nxt, g // DCH, g % DCH)
            etr_group(pecol, e_row, TG - 1)

            z = small.tile([1, 1], F32, tag="z")
            nc.vector.tensor_reduce(
                z, zpart, axis=mybir.AxisListType.X, op=mybir.AluOpType.add
            )
            nc.vector.tensor_copy(zall[:, b : b + 1], z)

            ecol = small.tile([P, TCH], MMDT, tag="ecol")
            nc.vector.tensor_copy(ecol, pecol[:, 0:TCH])

            # unnormalized attention weights out (host divides by Z)
            nc.sync.dma_start(out=attn_out[b : b + 1, :], in_=e_row)

            # context: sum_t e[t] * values[t, :] (unnormalized; host divides).
            # For b=0, batch 1's remaining transposes ride between context
            # chunks so their weight loads hide under the context streams.
            cx = psc.tile([1, D], F32, tag="sc", name=f"cx{b}")
            for ti in range(TCH):
                nc.tensor.matmul(
                    cx, ecol[:, ti : ti + 1], vnat[:, ti, :],
                    start=(ti == 0), stop=(ti == TCH - 1),
                )
                if b == 0 and nxt and ti % 4 == 3:
                    base = 4 + 3 * (ti // 4)
                    for g in range(base, base + 3):
                        tr_group(1, vnat_nxt, vT_nxt, g // DCH, g % DCH)
            ctx_sb = small.tile([1, D], F32, tag="ctx_sb", bufs=1)
            nc.scalar.activation(ctx_sb, cx, AF.Copy)
            nc.sync.dma_start(out=ctx_out[b : b + 1, :], in_=ctx_sb)

            if nxt:
                vnat_cur, vT_cur = vnat_nxt, vT_nxt

        nc.sync.dma_start(out=z_out, in_=zall)

    nc.compile()
    return nc


def _get_nc():
    if "nc" not in _CACHE:
        _CACHE["nc"] = build()
    return _CACHE["nc"]


def kernel(values, query, W1, b1, W2, b2, V, bV):
    nc = _get_nc()
    values = np.asarray(values, dtype=np.float32)
    query = np.asarray(query, dtype=np.float32)
    shared = {
        "W1": np.ascontiguousarray(W1, dtype=np.float32),
        "W2": np.ascontiguousarray(W2, dtype=np.float32),
        "b1": np.ascontiguousarray(b1, dtype=np.float32),
        "b2": np.ascontiguousarray(b2, dtype=np.float32),
        "V": np.ascontiguousarray(V, dtype=np.float32),
    }
    in_maps = []
    for c in range(NCORES):
        sl = slice(c * BL, (c + 1) * BL)
        in_maps.append(
            {
                "values_l": np.ascontiguousarray(values[sl]),
                "query_l": np.ascontiguousarray(query[sl]),
                **shared,
            }
        )
    res = run_bass_kernel_spmd(nc, in_maps, core_ids=list(range(NCORES)))
    context = np.concatenate([res.results[c]["ctx_out"] for c in range(NCORES)], 0)
    attn = np.concatenate([res.results[c]["attn_out"] for c in range(NCORES)], 0)
    zs = np.concatenate([res.results[c]["z_out"][0] for c in range(NCORES)], 0)
    context = context / zs[:, None]
    attn = attn / zs[:, None]
    return context, attn.reshape(B, T, 1)


# revision 15
# speedup vs baseline: 1.0382x; 1.0382x over previous
"""Bahdanau attention kernel for Trainium2, 8-core SPMD data-parallel over batch.

Problem shapes (hardcoded): values [32,2048,512] f32, query [32,512],
W1/W2 [512,512], b1/b2 [512], V [512,1], bV [1].

reference:
    proj_v = values @ W1 + b1            [B,T,U]
    proj_q = (query @ W2 + b2)[:,None,:] [B,1,U]
    score  = tanh(proj_v+proj_q) @ V + bV
    a      = softmax(score, axis=T)
    ctx    = sum_t a * values            [B,D]
    return ctx, a

Per-core plan (4 batches each):
  - load values[b] natural [t,d] -> SBUF (fast contiguous DMA)
  - PE-transpose 128x128 tiles -> vT [d,t] (PSUM->SBUF via DVE copies)
  - mm1: pv^T[u,t] = sum_d W1[d,u] * vT[d,t]  (fp32r, N=512, full PE rate)
  - ACT: s[u,t] = tanh(pv^T + beta[u,b]) where beta = (q@W2)+b1+b2 per batch
  - V-dot: score[1,t] = sum_u V[u]*s[u,t] (M=1 matmuls, accumulate over u-chunks)
  - ACT: e = exp(score) row-wise, with fused accumulated Z partials
  - PE-transpose e row -> e column layout [128,16]
  - context: sum_tc ecol[:,tc].T @ vnat[:,tc,:] -> [1,512]; scale by 1/Z
  - bV is skipped: softmax is shift-invariant, so it cancels exactly.
"""

import sys

if "/opt/trn_rl_repo" not in sys.path:
    sys.path.insert(0, "/opt/trn_rl_repo")

from contextlib import ExitStack

import numpy as np

import concourse.bacc as bacc
import concourse.tile as tile
from concourse import mybir
from concourse.bass_utils import run_bass_kernel_spmd
from concourse.masks import make_identity

F32 = mybir.dt.float32
MMDT = mybir.dt.float32r  # streaming dtype for matmuls (full PE rate at N>=256)
AF = mybir.ActivationFunctionType

B, T, D, U = 32, 2048, 512, 512
NCORES = 8
BL = B // NCORES  # batches per core
P = 128
DCH = D // P  # 4 d-chunks
UCH = U // P  # 4 u-chunks
TCH = T // P  # 16 t-chunks
TGS = 512     # matmul free-dim block over t
TG = T // TGS  # 4 t-blocks

_CACHE = {}


def _mm(ap):
    return ap.bitcast(MMDT)


def build(debug=False):
    nc = bacc.Bacc("TRN2", target_bir_lowering=False, debug=debug)

    vals = nc.dram_tensor("values_l", [BL, T, D], F32, kind="ExternalInput").ap()
    qry = nc.dram_tensor("query_l", [BL, D], F32, kind="ExternalInput").ap()
    w1 = nc.dram_tensor("W1", [D, U], F32, kind="ExternalInput").ap()
    w2 = nc.dram_tensor("W2", [D, U], F32, kind="ExternalInput").ap()
    b1 = nc.dram_tensor("b1", [U], F32, kind="ExternalInput").ap()
    b2 = nc.dram_tensor("b2", [U], F32, kind="ExternalInput").ap()
    vv = nc.dram_tensor("V", [U, 1], F32, kind="ExternalInput").ap()
    ctx_out = nc.dram_tensor("ctx_out", [BL, D], F32, kind="ExternalOutput").ap()
    attn_out = nc.dram_tensor("attn_out", [BL, T], F32, kind="ExternalOutput").ap()
    z_out = nc.dram_tensor("z_out", [1, BL], F32, kind="ExternalOutput").ap()

    with ExitStack() as ctx:
        tc = ctx.enter_context(tile.TileContext(nc))

        consts = ctx.enter_context(tc.tile_pool(name="consts", bufs=1))
        vnat_p = ctx.enter_context(tc.tile_pool(name="vnat", bufs=2))
        vT_p = ctx.enter_context(tc.tile_pool(name="vT", bufs=2))
        s_p = ctx.enter_context(tc.tile_pool(name="s", bufs=1))
        small = ctx.enter_context(tc.tile_pool(name="small", bufs=2))
        ptr = ctx.enter_context(tc.tile_pool(name="ptr", bufs=2, space="PSUM"))
        ppv = ctx.enter_context(tc.tile_pool(name="ppv", bufs=4, space="PSUM"))
        psc = ctx.enter_context(tc.tile_pool(name="psc", bufs=2, space="PSUM"))

        ident = consts.tile([P, P], F32)
        make_identity(nc, ident)
        identr = consts.tile([P, P], MMDT)
        nc.vector.tensor_copy(identr, ident)

        # small consts first (cheap DMA issues; setup matmuls need them early)
        qsb = consts.tile([BL, D], F32)
        nc.sync.dma_start(out=qsb, in_=qry)
        b1row = consts.tile([1, U], F32)
        nc.sync.dma_start(out=b1row, in_=b1.rearrange("(one u) -> one u", one=1))
        b2row = consts.tile([1, U], F32)
        nc.sync.dma_start(out=b2row, in_=b2.rearrange("(one u) -> one u", one=1))
        b12row = consts.tile([1, U], F32)
        nc.vector.tensor_add(b12row, b1row, b2row)
        vrow = consts.tile([1, U], F32)
        nc.sync.dma_start(out=vrow, in_=vv.rearrange("u one -> one u"))

        def load_vnat(b, name, nchunks=4):
            # values[b] partitioned p-major: partition p holds rows 16p..16p+15,
            # so each partition's span is one contiguous 32KB HBM block.
            # Chunked so the first transposes can start before the whole 4MB lands.
            vnat = vnat_p.tile([P, TCH, D], MMDT, tag="vnat", name=name)
            src_ap = vals[b].rearrange("(p t) d -> p t d", p=P).bitcast(MMDT)
            step = TCH // nchunks
            for c in range(nchunks):
                nc.sync.dma_start(
                    out=vnat[:, c * step : (c + 1) * step, :],
                    in_=src_ap[:, c * step : (c + 1) * step, :],
                )
            return vnat

        vnat0 = load_vnat(0, "vnat_first", nchunks=8)

        # W1 loaded with an f32r-casting SWDGE DMA; W2 stays f32 for the
        # (tiny) f32 beta matmul.
        W1r = consts.tile([P, DCH, U], MMDT)
        nc.gpsimd.dma_start(out=W1r, in_=w1.rearrange("(dc p) u -> p dc u", p=P))
        W2sb = consts.tile([P, DCH, U], F32)
        nc.sync.dma_start(out=W2sb, in_=w2.rearrange("(dc p) u -> p dc u", p=P))

        Vsb = consts.tile([P, UCH], MMDT)
        b12c = consts.tile([P, UCH], F32)
        qTc = consts.tile([P, DCH, BL], F32)
        beta = consts.tile([P, UCH, BL], F32)
        zall = consts.tile([1, BL], F32)

        def setup_small():
            # V, b1+b2, q transposed to column layouts; beta = (q@W2)^T + b1 + b2.
            ps_v = ptr.tile([P, TGS], F32, tag="tr", name="ps_v")
            for i in range(UCH):
                nc.tensor.matmul(
                    ps_v[:, i : i + 1], vrow[0:1, i * P : (i + 1) * P],
                    ident[0:1, 0:1], is_transpose=True,
                    start=(i == 0), stop=(i == UCH - 1),
                )
            nc.vector.tensor_copy(Vsb, ps_v[:, 0:UCH])

            ps_b = ptr.tile([P, TGS], F32, tag="tr", name="ps_b")
            for i in range(UCH):
                nc.tensor.matmul(
                    ps_b[:, i : i + 1], b12row[0:1, i * P : (i + 1) * P],
                    ident[0:1, 0:1], is_transpose=True,
                    start=(i == 0), stop=(i == UCH - 1),
                )
            nc.vector.tensor_copy(b12c, ps_b[:, 0:UCH])

            ps_q = ptr.tile([P, TGS], F32, tag="tr", name="ps_q")
            for dc in range(DCH):
                nc.tensor.matmul(
                    ps_q[:, dc * BL : (dc + 1) * BL], qsb[0:BL, dc * P : (dc + 1) * P],
                    ident[0:BL, 0:BL], is_transpose=True,
                    start=(dc == 0), stop=(dc == DCH - 1),
                )
            nc.vector.tensor_copy(qTc, ps_q[:, 0 : DCH * BL].rearrange("p (dc b) -> p dc b", dc=DCH))

            for uc in range(UCH):
                pq = ppv.tile([P, BL], F32, tag="pv", name=f"pq{uc}")
                for dc in range(DCH):
                    nc.tensor.matmul(
                        pq, W2sb[:, dc, uc * P : (uc + 1) * P], qTc[:, dc, :],
                        start=(dc == 0), stop=(dc == DCH - 1),
                    )
                nc.vector.tensor_scalar_add(beta[:, uc, :], pq, b12c[:, uc : uc + 1])

        def tr_group(b, vnat, vT, tg, dc):
            # one 512-wide column of the values transpose: 4 PE transposes + copy
            tr = ptr.tile([P, TGS], MMDT, tag="tr", name=f"tr{b}_{dc}_{tg}")
            for t4 in range(4):
                ti = tg * 4 + t4
                nc.tensor.matmul(
                    tr[:, t4 * P : (t4 + 1) * P],
                    vnat[:, ti, dc * P : (dc + 1) * P],
                    identr, is_transpose=True,
                    start=(t4 == 0), stop=(t4 == 3),
                )
            nc.vector.tensor_copy(vT[:, dc, tg * TGS : (tg + 1) * TGS], tr)

        def mm1_group(b, s, vT, tg, uc):
            pv = ppv.tile([P, TGS], F32, tag="pv")
            for dc in range(DCH):
                nc.tensor.matmul(
                    pv, W1r[:, dc, uc * P : (uc + 1) * P],
                    vT[:, dc, tg * TGS : (tg + 1) * TGS],
                    start=(dc == 0), stop=(dc == DCH - 1),
                )
            nc.scalar.activation(
                s[:, uc, tg * TGS : (tg + 1) * TGS], pv, AF.Tanh,
                bias=beta[:, uc, b : b + 1],
            )

        def vdot_exp(b, s, e_row, zpart, tg):
            sc = psc.tile([1, TGS], F32, tag="sc", name=f"sc{b}_{tg}")
            for uc in range(UCH):
                nc.tensor.matmul(
                    sc, Vsb[:, uc : uc + 1],
                    s[:, uc, tg * TGS : (tg + 1) * TGS],
                    start=(uc == 0), stop=(uc == UCH - 1),
                )
            nc.scalar.activation(
                e_row[:, tg * TGS : (tg + 1) * TGS], sc, AF.Exp,
                accum_out=zpart[:, tg : tg + 1],
            )

        setup_small()

        vnat_cur = vnat0
        vT_cur = vT_p.tile([P, DCH, T], MMDT, tag="vT", name="vT_first")
        for tg in range(TG):
            for dc in range(DCH):
                tr_group(0, vnat0, vT_cur, tg, dc)

        for b in range(BL):
            vnat, vT = vnat_cur, vT_cur
            nxt = b + 1 < BL
            if nxt:
                vnat_nxt = load_vnat(b + 1, f"vnat{b+1}")
                vT_nxt = vT_p.tile([P, DCH, T], MMDT, tag="vT", name=f"vT{b+1}")

            s = s_p.tile([P, UCH, T], MMDT, tag="s")
            e_row = small.tile([1, T], F32, tag="e_row", bufs=1)
            zpart = small.tile([1, TG], F32, tag="zpart")
            pecol = ppv.tile([P, TGS], F32, tag="pv", name=f"pecol{b}")

            for tg in range(TG):
                for uc in range(UCH):
                    mm1_group(b, s, vT, tg, uc)

            # next batch's transposes fill the PE while the softmax chain runs
            if nxt:
                for tg in range(TG):
                    for dc in range(DCH):
                        tr_group(b + 1, vnat_nxt, vT_nxt, tg, dc)

            for tg in range(TG):
                vdot_exp(b, s, e_row, zpart, tg)

            z = small.tile([1, 1], F32, tag="z")
            nc.vector.tensor_reduce(
                z, zpart, axis=mybir.AxisListType.X, op=mybir.AluOpType.add
            )
            nc.vector.tensor_copy(zall[:, b : b + 1], z)

            # e row -> column layout [p, tc]
            for ti in range(TCH):
                nc.tensor.matmul(
                    pecol[:, ti : ti + 1], e_row[0:1, ti * P : (ti + 1) * P],
                    ident[0:1, 0:1], is_transpose=True,
                    start=(ti == 0), stop=(ti == TCH - 1),
                )
            ecol = small.tile([P, TCH], MMDT, tag="ecol")
            nc.vector.tensor_copy(ecol, pecol[:, 0:TCH])

            # unnormalized attention weights out (host divides by Z)
            nc.sync.dma_start(out=attn_out[b : b + 1, :], in_=e_row)

            # context: sum_t e[t] * values[t, :] (unnormalized; host divides)
            cx = psc.tile([1, D], F32, tag="sc", name=f"cx{b}")
            for ti in range(TCH):
                nc.tensor.matmul(
                    cx, ecol[:, ti : ti + 1], vnat[:, ti, :],
                    start=(ti == 0), stop=(ti == TCH - 1),
                )
            ctx_sb = small.tile([1, D], F32, tag="ctx_sb", bufs=1)
            nc.scalar.activation(ctx_sb, cx, AF.Copy)
            nc.sync.dma_start(out=ctx_out[b : b + 1, :], in_=ctx_sb)

            if nxt:
                vnat_cur, vT_cur = vnat_nxt, vT_nxt

        nc.sync.dma_start(out=z_out, in_=zall)

    nc.compile()
    return nc


def _get_nc():
    if "nc" not in _CACHE:
        _CACHE["nc"] = build()
    return _CACHE["nc"]


def kernel(values, query, W1, b1, W2, b2, V, bV):
    nc = _get_nc()
    values = np.asarray(values, dtype=np.float32)
    query = np.asarray(query, dtype=np.float32)
    shared = {
        "W1": np.ascontiguousarray(W1, dtype=np.float32),
        "W2": np.ascontiguousarray(W2, dtype=np.float32),
        "b1": np.ascontiguousarray(b1, dtype=np.float32),
        "b2": np.ascontiguousarray(b2, dtype=np.float32),
        "V": np.ascontiguousarray(V, dtype=np.float32),
    }
    in_maps = []
    for c in range(NCORES):
        sl = slice(c * BL, (c + 1) * BL)
        in_maps.append(
            {
                "values_l": np.ascontiguousarray(values[sl]),
                "query_l": np.ascontiguousarray(query[sl]),
                **shared,
            }
        )
    res = run_bass_kernel_spmd(nc, in_maps, core_ids=list(range(NCORES)))
    context = np.concatenate([res.results[c]["ctx_out"] for c in range(NCORES)], 0)
    attn = np.concatenate([res.results[c]["attn_out"] for c in range(NCORES)], 0)
    zs = np.concatenate([res.results[c]["z_out"][0] for c in range(NCORES)], 0)
    context = context / zs[:, None]
    attn = attn / zs[:, None]
    # on-chip t-order is k = j*128 + p for t = 16p + j; undo it
    attn = attn.reshape(B, TCH, P).transpose(0, 2, 1).reshape(B, T, 1)
    return context, attn
